# revision 32
# baseline (speedup 1.0000x reference)
"""Cluster-memory cross-entropy loss on 8 Trainium2 NeuronCores.

Problem: loss = -mean_b log_softmax(normalize(inputs) @ features.T / T)[b, targets[b]]
  inputs [512, 256] f32, features [65536, 256] f32 (unit rows), targets [512] int.

Strategy: shard the 65536 cluster columns across 8 cores (8192 each). Each
core computes its shard's per-row sum(exp(logits)):
  TensorE: logit pieces [128 batch, <=2048 clusters] in PSUM (bf16 matmuls,
           fp32 PSUM accumulation)
  ScalarE: exp PSUM -> SBUF (bf16)
  VectorE: (add halves for 2048-wide pieces) + free-dim reduce -> f32 partials
The host combines the 8 partial sum-exp vectors, computes log, and takes
the target logits with a 512-row gather-dot in exact fp32 — bf16 noise only
enters through logZ, where it averages out across 65536 clusters (measured
~3e-6 relative loss error).

Per-core input is a single [128, 17408] bf16 blob: xt first (m-major), then
feature slabs (two 512-col slabs, then seven 1024-col slabs) laid out in
SBUF order so every DMA is one contiguous per-partition range. The early
pieces are narrow (512/1024 cols) so ScalarE starts as soon as the first
256 KB slab lands; steady-state pieces are 2048 cols (full PSUM half).
"""

import numpy as np
import ml_dtypes

import concourse.bass as bass
import concourse.mybir as mybir
import concourse.tile as tile
from concourse import bacc
from concourse.bass_utils import run_bass_kernel_spmd

B, N, D, TEMP = 512, 65536, 256, 0.05
NCORES = 8
NSH = N // NCORES      # 8192 cluster columns per core
MT = B // 128          # 4 batch tiles of 128
KT = D // 128          # 2 contraction chunks of 128

# feature slabs (DMA units): two 512-col, then seven 1024-col
SLAB_WIDTHS = [512, 512] + [1024] * 7
SLAB_STARTS = np.cumsum([0] + SLAB_WIDTHS).tolist()  # [0,512,1024,2048,...,8192]
assert SLAB_STARTS[-1] == NSH

XT_W = KT * B                       # 1024 blob cols for xt
BLOB_W = XT_W + KT * NSH            # 17408

F32 = mybir.dt.float32
BF16 = mybir.dt.bfloat16

ACC_SLOTS = 7


def piece_plan():
    """(m, col_lo, width, acc_slot) in emission order. Group 0 (cols 0-2047)
    is split finer for pipeline ramp-up; groups 1-3 use full 2048 pieces;
    the final piece is narrow so the tail chain (exp+reduce+DMA) is short."""
    plan = []
    # ramp: consume resident slabs across all batch tiles before needing
    # the next slab (column-major), so ScalarE never waits on the DMA
    plan.append((0, 0, 512, 0))
    plan.append((0, 512, 512, 1))
    for m in range(1, MT):
        plan.append((m, 0, 1024, 0))
    for m in range(MT):
        plan.append((m, 1024, 1024, 2))
    for jg in range(1, 4):
        for m in range(MT):
            if jg == 3 and m == MT - 1:
                plan.append((m, jg * 2048, 1024, 2 + jg))
                plan.append((m, jg * 2048 + 1024, 1024, 6))
            else:
                plan.append((m, jg * 2048, 2048, 2 + jg))
    return plan


def build_nc():
    nc = bacc.Bacc(target_bir_lowering=False, enable_partition_id=False)
    data = nc.declare_dram_parameter("data", [128, BLOB_W], BF16, isOutput=False)
    out = nc.declare_dram_parameter("out", [128, MT * ACC_SLOTS], F32, isOutput=True)

    with tile.TileContext(nc) as tc:
        with (
            tc.tile_pool(name="xt_pool", bufs=1) as xt_pool,
            tc.tile_pool(name="slab_pool", bufs=len(SLAB_WIDTHS)) as slab_pool,
            tc.tile_pool(name="psum", bufs=2, space="PSUM") as psum_pool,
            tc.tile_pool(name="expv", bufs=4) as exp_pool,
            tc.tile_pool(name="evh", bufs=4) as evh_pool,
            tc.tile_pool(name="acc", bufs=1) as acc_pool,
        ):
            # xt goes over the second HWDGE ring (ACT sequencer) so it
            # transfers concurrently with the first feature slab
            xt_t = xt_pool.tile([128, MT, KT, 128], BF16)
            nc.scalar.dma_start(
                out=xt_t[:],
                in_=data[:, 0:XT_W].rearrange("p (mt k c) -> p mt k c", mt=MT, k=KT),
            )
            acc = acc_pool.tile([128, MT * ACC_SLOTS], F32)


            # all slabs stay resident; DMAs issue back-to-back in consumption order
            slabs = []
            for sl, w in enumerate(SLAB_WIDTHS):
                st = slab_pool.tile([128, KT, w], BF16, tag=f"slab{w}")
                off = XT_W + KT * SLAB_STARTS[sl]
                eng = nc.sync if sl % 2 == 0 else nc.scalar
                eng.dma_start(
                    out=st[:],
                    in_=data[:, off : off + KT * w].rearrange(
                        "p (k n) -> p k n", k=KT
                    ),
                )
                slabs.append(st)

            def find_slab(col):
                for sl in range(len(SLAB_WIDTHS)):
                    if SLAB_STARTS[sl] <= col < SLAB_STARTS[sl + 1]:
                        return sl
                raise AssertionError(col)

            for m, lo, w, slot in piece_plan():
                ps = psum_pool.tile([128, 2048], F32, tag="ps")
                for g in range(lo, lo + w, 512):
                    sl = find_slab(g)
                    for k in range(KT):
                        nc.tensor.matmul(
                            ps[:, g - lo : g - lo + 512],
                            lhsT=xt_t[:, m, k, :],
                            rhs=slabs[sl][:, k, g - SLAB_STARTS[sl] : g - SLAB_STARTS[sl] + 512],
                            start=(k == 0),
                            stop=(k == KT - 1),
                        )
                ev = exp_pool.tile([128, w], BF16, tag="ev")
                nc.scalar.activation(
                    ev[:], ps[:, :w], mybir.ActivationFunctionType.Exp
                )
                if w == 2048:
                    evh = evh_pool.tile([128, 1024], BF16, tag="evh")
                    nc.vector.tensor_add(evh[:], ev[:, :1024], ev[:, 1024:])
                    red_in = evh[:]
                else:
                    red_in = ev[:]
                col = m * ACC_SLOTS + slot
                nc.vector.reduce_sum(
                    acc[:, col : col + 1], red_in, axis=mybir.AxisListType.X
                )
            # ship the bulk of the accumulators while the last two pieces
            # finish; only the final two columns ride the tail chain
            split = (MT - 1) * ACC_SLOTS + 5
            nc.sync.dma_start(out=out[:, :split], in_=acc[:, :split])
            nc.sync.dma_start(out=out[:, split:], in_=acc[:, split:])
    nc.compile()
    return nc


_NC_CACHE = {}


def _get_nc():
    if "nc" not in _NC_CACHE:
        _NC_CACHE["nc"] = build_nc()
    return _NC_CACHE["nc"]


def prep_inputs(inputs, features):
    """Host-side data prep: normalize+scale x, transpose both into the
    SBUF-resident layouts, pack per-core bf16 blobs so every DMA is
    contiguous."""
    xn = inputs / np.linalg.norm(inputs, axis=1, keepdims=True)
    xs = (xn / TEMP).astype(np.float32)
    # xt[p, mt, k, c] = xs[mt*128+c, k*128+p] -> flat [128, 1024], m-major
    xt_flat = xs.reshape(MT, 128, KT, 128).transpose(3, 0, 2, 1).reshape(128, XT_W)
    blobs = []
    for c in range(NCORES):
        fc = features[c * NSH:(c + 1) * NSH]  # [8192, 256]
        # per slab: [p, k, n] = fc[start+n, k*128+p], concatenated
        parts = [xt_flat]
        for sl, w in enumerate(SLAB_WIDTHS):
            s = SLAB_STARTS[sl]
            parts.append(
                fc[s : s + w].reshape(w, KT, 128).transpose(2, 1, 0).reshape(128, KT * w)
            )
        blob = np.concatenate(parts, axis=1).astype(ml_dtypes.bfloat16)
        blobs.append(np.ascontiguousarray(blob))
    return xs, blobs


def run_cores(blobs, **kwargs):
    nc = _get_nc()
    in_maps = [{"data": blobs[c]} for c in range(NCORES)]
    return run_bass_kernel_spmd(nc, in_maps, list(range(NCORES)), **kwargs)


def combine(results, xs, features, targets):
    # sum only the acc slots the device actually wrote (per piece_plan)
    cols = sorted({m * ACC_SLOTS + slot for m, _, _, slot in piece_plan()})
    sumexp = np.zeros(B, dtype=np.float64)
    for c in range(NCORES):
        o = results[c]["out"].astype(np.float64)  # [128, MT*ACC_SLOTS]
        per_row = o.reshape(128, MT, ACC_SLOTS)
        for m in range(MT):
            mc = [s for s in range(ACC_SLOTS) if m * ACC_SLOTS + s in cols]
            sumexp[m * 128:(m + 1) * 128] += per_row[:, m, mc].sum(axis=1)
    logz = np.log(sumexp)
    t_logit = (xs * features[targets]).sum(axis=1).astype(np.float64)
    loss = np.mean(logz - t_logit)
    return np.float32(loss)


def kernel(inputs, ema_inputs, targets, features):
    inputs = np.asarray(inputs, dtype=np.float32)
    features = np.asarray(features, dtype=np.float32)
    targets = np.asarray(targets)
    xs, blobs = prep_inputs(inputs, features)
    results = run_cores(blobs).results
    return combine(results, xs, features, targets)


# revision 33
# speedup vs baseline: 1.0454x; 1.0454x over previous
"""Cluster-memory cross-entropy loss on 8 Trainium2 NeuronCores.

Problem: loss = -mean_b log_softmax(normalize(inputs) @ features.T / T)[b, targets[b]]
  inputs [512, 256] f32, features [65536, 256] f32 (unit rows), targets [512] int.

Strategy: shard the 65536 cluster columns across 8 cores (8192 each). Each
core computes its shard's per-row sum(exp(logits)):
  TensorE: logit pieces [128 batch, <=2048 clusters] in PSUM (bf16 matmuls,
           fp32 PSUM accumulation)
  ScalarE: exp PSUM -> SBUF (bf16)
  VectorE: (add halves for 2048-wide pieces) + free-dim reduce -> f32 partials
The host combines the 8 partial sum-exp vectors, computes log, and takes
the target logits with a 512-row gather-dot in exact fp32 — bf16 noise only
enters through logZ, where it averages out across 65536 clusters (measured
~3e-6 relative loss error).

Per-core input is a single [128, 17408] bf16 blob: xt first (m-major), then
feature slabs (two 512-col slabs, then seven 1024-col slabs) laid out in
SBUF order so every DMA is one contiguous per-partition range. The early
pieces are narrow (512/1024 cols) so ScalarE starts as soon as the first
256 KB slab lands; steady-state pieces are 2048 cols (full PSUM half).
"""

import numpy as np
import ml_dtypes

import concourse.bass as bass
import concourse.mybir as mybir
import concourse.tile as tile
from concourse import bacc
from concourse.bass_utils import run_bass_kernel_spmd

B, N, D, TEMP = 512, 65536, 256, 0.05
NCORES = 8
NSH = N // NCORES      # 8192 cluster columns per core
MT = B // 128          # 4 batch tiles of 128
KT = D // 128          # 2 contraction chunks of 128

# feature slabs (DMA units): two 512-col, then seven 1024-col
SLAB_WIDTHS = [512, 512] + [1024] * 7
SLAB_STARTS = np.cumsum([0] + SLAB_WIDTHS).tolist()  # [0,512,1024,2048,...,8192]
assert SLAB_STARTS[-1] == NSH

XT_W = KT * B                       # 1024 blob cols for xt
BLOB_W = XT_W + KT * NSH            # 17408

F32 = mybir.dt.float32
BF16 = mybir.dt.bfloat16

ACC_SLOTS = 7


def piece_plan():
    """(m, col_lo, width, acc_slot) in emission order. Group 0 (cols 0-2047)
    is split finer for pipeline ramp-up; groups 1-3 use full 2048 pieces;
    the final piece is narrow so the tail chain (exp+reduce+DMA) is short."""
    plan = []
    # ramp: consume resident slabs across all batch tiles before needing
    # the next slab (column-major), so ScalarE never waits on the DMA
    plan.append((0, 0, 512, 0))
    plan.append((0, 512, 512, 1))
    for m in range(1, MT):
        plan.append((m, 0, 1024, 0))
    for m in range(MT):
        plan.append((m, 1024, 1024, 2))
    for jg in range(1, 4):
        for m in range(MT):
            if jg == 3 and m == MT - 1:
                plan.append((m, jg * 2048, 1024, 2 + jg))
                plan.append((m, jg * 2048 + 1024, 1024, 6))
            else:
                plan.append((m, jg * 2048, 2048, 2 + jg))
    return plan


def build_nc():
    nc = bacc.Bacc(target_bir_lowering=False, enable_partition_id=False)
    data = nc.declare_dram_parameter("data", [128, BLOB_W], BF16, isOutput=False)
    out = nc.declare_dram_parameter("out", [128, MT * ACC_SLOTS], F32, isOutput=True)

    with tile.TileContext(nc) as tc:
        with (
            tc.tile_pool(name="xt_pool", bufs=1) as xt_pool,
            tc.tile_pool(name="slab_pool", bufs=len(SLAB_WIDTHS)) as slab_pool,
            tc.tile_pool(name="psum", bufs=2, space="PSUM") as psum_pool,
            tc.tile_pool(name="expv", bufs=4) as exp_pool,
            tc.tile_pool(name="evh", bufs=4) as evh_pool,
            tc.tile_pool(name="acc", bufs=1) as acc_pool,
        ):
            # xt goes over the second HWDGE ring (ACT sequencer) so it
            # transfers concurrently with the first feature slab
            xt_t = xt_pool.tile([128, MT, KT, 128], BF16)
            nc.scalar.dma_start(
                out=xt_t[:],
                in_=data[:, 0:XT_W].rearrange("p (mt k c) -> p mt k c", mt=MT, k=KT),
            )
            acc = acc_pool.tile([128, MT * ACC_SLOTS], F32)


            # all slabs stay resident; DMAs issue back-to-back in consumption order
            slabs = []
            for sl, w in enumerate(SLAB_WIDTHS):
                st = slab_pool.tile([128, KT, w], BF16, tag=f"slab{w}")
                off = XT_W + KT * SLAB_STARTS[sl]
                nc.sync.dma_start(
                    out=st[:],
                    in_=data[:, off : off + KT * w].rearrange(
                        "p (k n) -> p k n", k=KT
                    ),
                )
                slabs.append(st)

            def find_slab(col):
                for sl in range(len(SLAB_WIDTHS)):
                    if SLAB_STARTS[sl] <= col < SLAB_STARTS[sl + 1]:
                        return sl
                raise AssertionError(col)

            for m, lo, w, slot in piece_plan():
                ps = psum_pool.tile([128, 2048], F32, tag="ps")
                for g in range(lo, lo + w, 512):
                    sl = find_slab(g)
                    for k in range(KT):
                        nc.tensor.matmul(
                            ps[:, g - lo : g - lo + 512],
                            lhsT=xt_t[:, m, k, :],
                            rhs=slabs[sl][:, k, g - SLAB_STARTS[sl] : g - SLAB_STARTS[sl] + 512],
                            start=(k == 0),
                            stop=(k == KT - 1),
                        )
                ev = exp_pool.tile([128, w], BF16, tag="ev")
                nc.scalar.activation(
                    ev[:], ps[:, :w], mybir.ActivationFunctionType.Exp
                )
                if w == 2048:
                    evh = evh_pool.tile([128, 1024], BF16, tag="evh")
                    nc.vector.tensor_add(evh[:], ev[:, :1024], ev[:, 1024:])
                    red_in = evh[:]
                else:
                    red_in = ev[:]
                col = m * ACC_SLOTS + slot
                nc.vector.reduce_sum(
                    acc[:, col : col + 1], red_in, axis=mybir.AxisListType.X
                )
            # ship the bulk of the accumulators while the last two pieces
            # finish; only the final two columns ride the tail chain
            split = (MT - 1) * ACC_SLOTS + 5
            nc.sync.dma_start(out=out[:, :split], in_=acc[:, :split])
            nc.sync.dma_start(out=out[:, split:], in_=acc[:, split:])
    nc.compile()
    return nc


_NC_CACHE = {}


def _get_nc():
    if "nc" not in _NC_CACHE:
        _NC_CACHE["nc"] = build_nc()
    return _NC_CACHE["nc"]


def prep_inputs(inputs, features):
    """Host-side data prep: normalize+scale x, transpose both into the
    SBUF-resident layouts, pack per-core bf16 blobs so every DMA is
    contiguous."""
    xn = inputs / np.linalg.norm(inputs, axis=1, keepdims=True)
    xs = (xn / TEMP).astype(np.float32)
    # xt[p, mt, k, c] = xs[mt*128+c, k*128+p] -> flat [128, 1024], m-major
    xt_flat = xs.reshape(MT, 128, KT, 128).transpose(3, 0, 2, 1).reshape(128, XT_W)
    blobs = []
    for c in range(NCORES):
        fc = features[c * NSH:(c + 1) * NSH]  # [8192, 256]
        # per slab: [p, k, n] = fc[start+n, k*128+p], concatenated
        parts = [xt_flat]
        for sl, w in enumerate(SLAB_WIDTHS):
            s = SLAB_STARTS[sl]
            parts.append(
                fc[s : s + w].reshape(w, KT, 128).transpose(2, 1, 0).reshape(128, KT * w)
            )
        blob = np.concatenate(parts, axis=1).astype(ml_dtypes.bfloat16)
        blobs.append(np.ascontiguousarray(blob))
    return xs, blobs


def run_cores(blobs, **kwargs):
    nc = _get_nc()
    in_maps = [{"data": blobs[c]} for c in range(NCORES)]
    return run_bass_kernel_spmd(nc, in_maps, list(range(NCORES)), **kwargs)


def combine(results, xs, features, targets):
    # sum only the acc slots the device actually wrote (per piece_plan)
    cols = sorted({m * ACC_SLOTS + slot for m, _, _, slot in piece_plan()})
    sumexp = np.zeros(B, dtype=np.float64)
    for c in range(NCORES):
        o = results[c]["out"].astype(np.float64)  # [128, MT*ACC_SLOTS]
        per_row = o.reshape(128, MT, ACC_SLOTS)
        for m in range(MT):
            mc = [s for s in range(ACC_SLOTS) if m * ACC_SLOTS + s in cols]
            sumexp[m * 128:(m + 1) * 128] += per_row[:, m, mc].sum(axis=1)
    logz = np.log(sumexp)
    t_logit = (xs * features[targets]).sum(axis=1).astype(np.float64)
    loss = np.mean(logz - t_logit)
    return np.float32(loss)


def kernel(inputs, ema_inputs, targets, features):
    inputs = np.asarray(inputs, dtype=np.float32)
    features = np.asarray(features, dtype=np.float32)
    targets = np.asarray(targets)
    xs, blobs = prep_inputs(inputs, features)
    results = run_cores(blobs).results
    return combine(results, xs, features, targets)
